# revision 19
# baseline (speedup 1.0000x reference)
"""Bahdanau attention kernel for Trainium2 (8 NeuronCores, batch-parallel).

Computes, for B=64, T=2048, E=512, H=128, D=1024:
    q   = dec @ W_w.T                      [B, H]
    k   = enc @ V_w.T + V_b                [B, T, H]
    erg = tanh(q + k) @ w_w[0] (+ w_b)     [B, T]
    erg = where(mask, -inf, erg)
    aw  = softmax(erg, axis=1)             [B, T]
    ctx = einsum('bte,bt->be', enc, aw)    [B, E]
returns (ctx, aw).

Sharding: data-parallel over batch, 8 batches per core; weights replicated.
w_b is dropped (softmax is shift-invariant). Softmax skips max-subtraction:
|erg| <= sum|w_w| ~ 11.4, so exp never overflows; masked entries get -1e30
which underflows exp to exactly 0.

The host ships encoder_outs twice in fp16 — natural [t, e] (context rhs) and
pre-transposed [e, t] (projection rhs) — same HBM bytes as one fp32 copy.
This removes all on-chip 128x128 PE transposes of enc (the dominant cost of
the v1 kernel: each matmul carries a ~125ns LDWEIGHTS + drain overhead, and
transposes were 512 of ~1100 matmuls). Small weights (V_wT, W_wT, decT, w)
and the additive mask (pre-scaled to -1e30, transposed to [t%128, c]) are
also pre-arranged on the host.

Per-core dataflow (batch b local):
  - k[h=128p, 512t] = sum_ec V_wT_f16[:,ec].T @ encT_f16[:,ec,g] (fp16, N=512)
  - tanh fused with per-partition bias q[h]+V_b[h] on ACT, PSUM source.
  - erg.T[t=128p, 1] per 128-t block via matmul(lhsT=tanh_f16_block, rhs=w_f16)
  - masked softmax on [128, 16] (t%128 x t//128) tiles; partition sums via
    ones-matmuls on PE (cheap, N=1).
  - ctx[1, 512] accumulated over 16 chunks: matmul(lhsT=alpha_f16_col,
    rhs=enc_f16_chunk).
  - aw output via PE transpose of alpha (fp32r) back to [c, t%128] rows.
Batches are software-pipelined: frontend(b) then backend(b-1).
"""

import numpy as np
from contextlib import ExitStack

import concourse.bacc as bacc
import concourse.bass as bass
import concourse.tile as tile
from concourse import mybir
from concourse.bass_utils import run_bass_kernel_spmd
from concourse.masks import make_identity

F32 = mybir.dt.float32
F32R = mybir.dt.float32r
F16 = mybir.dt.float16

B, T, E, H, D = 64, 2048, 512, 128, 1024
NCORES = 8
BL = B // NCORES          # 8 local batches per core
NC_T = T // 128           # 16 chunks of 128 along t
NG = T // 512             # 4 groups of 512 along t
NE = E // 128             # 4 e-chunks
ND = D // 128             # 8 d-chunks
NEG = -1.0e30


def _frontend(nc, pools, aps, b):
    """Load enc[b]/encT[b]/maskT[b]; project, tanh, erg -> erg_psum[b]."""
    (consts, encp, tanhp, smalls, ptp, pkp, pergp, _pctxp) = pools
    enc, encT, maskT_in, _oc, _oa = aps["io"]
    ident, vwT, qvb, w_sb, ones, ones_row, negfour = aps["w"]

    # t index mapping everywhere on-chip: t = 16*p + c (p = partition,
    # c = chunk). enc rows for one partition are then 16 KB contiguous in
    # DRAM (full-rate DMA descriptors); encT is shipped with its t columns
    # pre-permuted to match, so per-partition runs are 4 KB.
    enc_t = encp.tile([128, NC_T, E], F16, tag="enc", name=f"enc_t{b}")
    encT_t = encp.tile([128, NE, T], F16, tag="encT", name=f"encT_t{b}")
    encT_src = encT[b].rearrange("(ec p) t -> p ec t", p=128)
    for ec in range(NE):
        nc.sync.dma_start(out=encT_t[:, ec, :], in_=encT_src[:, ec, :])
    nc.sync.dma_start(out=enc_t, in_=enc[b].rearrange("(p c) e -> p c e", c=NC_T))

    maskT = smalls.tile([128, NC_T], F32, tag="maskT", name=f"maskT{b}")
    nc.sync.dma_start(out=maskT, in_=maskT_in[b])

    erg_ps = pergp.tile([128, NC_T], F32, tag="ergp", name=f"erg_ps{b}")
    tanh_ts = []
    for g in range(NG):
        # k[h, 512t] accumulated over e-chunks (fp16, N=512)
        kp = pkp.tile([128, 512], F32, tag="kp", name=f"kp{b}_{g}")
        for ec in range(NE):
            nc.tensor.matmul(
                kp,
                lhsT=vwT[:, ec, :],
                rhs=encT_t[:, ec, 512 * g : 512 * (g + 1)],
                start=(ec == 0),
                stop=(ec == NE - 1),
            )
        # tanh(k + q[h] + V_b[h]) on ACT straight from PSUM, fp16 out
        tanh_t = tanhp.tile([128, 512], F16, tag="tanh", name=f"tanh{b}_{g}")
        nc.scalar.activation(
            tanh_t, kp, mybir.ActivationFunctionType.Tanh, bias=qvb[:, b : b + 1]
        )
        tanh_ts.append(tanh_t)

        # erg lags two groups behind so tanh (ACT) is never on the PE
        # critical path; the last two groups are emitted by the caller
        # after backend(b-1)'s matmuls.
        if g >= 2:
            _erg_group(nc, erg_ps, tanh_ts[g - 2], w_sb, g - 2)

    return enc_t, erg_ps, maskT, tanh_ts


def _erg_group(nc, erg_ps, tanh_t, w_sb, g):
    for tb in range(4):
        col = 4 * g + tb
        nc.tensor.matmul(
            erg_ps[:, col : col + 1],
            lhsT=tanh_t[:, 128 * tb : 128 * (tb + 1)],
            rhs=w_sb,
            start=True,
            stop=True,
        )


def _softmax(nc, pools, aps, b, enc_t, erg_ps, maskT, tanh_ts):
    """Unnormalized masked softmax of batch b (DVE/ACT only): the -4 shift
    (folded into the host-side mask) keeps fp16 in range and cancels in the
    host-side division by the row sum."""
    (consts, encp, tanhp, smalls, ptp, pkp, pergp, pctxp) = pools
    ergm = smalls.tile([128, NC_T], F32, tag="ergm", name=f"ergm{b}")
    nc.vector.tensor_add(ergm, erg_ps, maskT)
    e_sb = smalls.tile([128, NC_T], F32, tag="e_sb", name=f"e_sb{b}")
    nc.scalar.activation(e_sb, ergm, mybir.ActivationFunctionType.Exp)
    alpha = smalls.tile([128, NC_T], F32R, tag="alpha", name=f"alpha{b}")
    nc.vector.tensor_scalar_mul(alpha, e_sb, 1.0)
    alpha16 = smalls.tile([128, NC_T], F16, tag="alpha16", name=f"alpha16_{b}")
    nc.vector.tensor_scalar_mul(alpha16, e_sb, 1.0)
    return alpha, alpha16


def _pe_backend(nc, pools, aps, b, enc_t, alpha, alpha16):
    """context(b) and outputs(b)."""
    (consts, encp, tanhp, smalls, ptp, pkp, pergp, pctxp) = pools
    _enc, _encT, _m, out_ctx, out_aw = aps["io"]
    ident, vwT, qvb, w_sb, ones, ones_row, negfour = aps["w"]

    # context: ctx[1, 512] += alpha_chunk.T @ enc_chunk over 16 chunks (fp16)
    ctx_ps = pctxp.tile([1, E], F32, tag="ctxp", name=f"ctx_ps{b}")
    for c in range(NC_T):
        nc.tensor.matmul(
            ctx_ps,
            lhsT=alpha16[:, c : c + 1],
            rhs=enc_t[:, c, :],
            start=(c == 0),
            stop=(c == NC_T - 1),
        )
    ctx_sb = smalls.tile([1, E], F32, tag="ctx_sb", name=f"ctx_sb{b}")
    nc.vector.tensor_copy(ctx_sb, ctx_ps)
    nc.sync.dma_start(out=out_ctx[b : b + 1, :], in_=ctx_sb)

    # attention weights out: transpose alpha back to [c, t%128] rows (fp32r)
    et_ps = ptp.tile([16, 128], F32R, tag="tp", name=f"et_ps{b}")
    nc.tensor.transpose(et_ps, alpha, ident)
    aw_sb = smalls.tile([16, 128], F32, tag="aw_sb", name=f"aw_sb{b}")
    nc.vector.tensor_copy(aw_sb, et_ps.bitcast(F32))
    nc.sync.dma_start(out=out_aw[b].rearrange("(c p) -> c p", p=128), in_=aw_sb)


def build_kernel(ctx, tc, aps):
    nc = tc.nc
    (enc, encT, maskT_in, decT, V_b, W_wT, w_in, out_ctx, out_aw) = aps["dram"]

    consts = ctx.enter_context(tc.tile_pool(name="consts", bufs=1))
    encp = ctx.enter_context(tc.tile_pool(name="encp", bufs=5))
    tanhp = ctx.enter_context(tc.tile_pool(name="tanhp", bufs=3))
    smalls = ctx.enter_context(tc.tile_pool(name="smalls", bufs=2))
    ptp = ctx.enter_context(tc.tile_pool(name="ptp", bufs=2, space="PSUM"))
    pkp = ctx.enter_context(tc.tile_pool(name="pkp", bufs=3, space="PSUM"))
    pergp = ctx.enter_context(tc.tile_pool(name="pergp", bufs=2, space="PSUM"))
    pctxp = ctx.enter_context(tc.tile_pool(name="pctxp", bufs=1, space="PSUM"))
    pools = (consts, encp, tanhp, smalls, ptp, pkp, pergp, pctxp)

    # ---- constants / setup ----
    ident32 = consts.tile([128, 128], F32)
    make_identity(nc, ident32)
    # gpsimd can't write f32r; round the identity through a DVE copy instead
    ident = consts.tile([128, 128], F32R)
    nc.vector.tensor_copy(ident, ident32)

    ones = consts.tile([128, 1], F32)
    nc.vector.memset(ones, 1.0)
    ones_row = consts.tile([1, 128], F32)
    nc.vector.memset(ones_row, 1.0)
    negfour = consts.tile([128, 1], F32)
    nc.vector.memset(negfour, -4.0)

    # host-pretransposed weights: V_wT [e, ec, h] fp16
    vwT = consts.tile([128, NE, 128], F16)
    nc.sync.dma_start(
        out=vwT, in_=aps["vwt_dram"].rearrange("(ec e) h -> e ec h", e=128)
    )

    # W_wT [d, dc, h] f32 and decT [d, dc, b] f32 for the q projection
    wwT = consts.tile([128, ND, 128], F32)
    nc.sync.dma_start(out=wwT, in_=W_wT.rearrange("(dc d) h -> d dc h", d=128))
    decT_sb = consts.tile([128, ND, BL], F32)
    nc.sync.dma_start(out=decT_sb, in_=decT.rearrange("(dc d) b -> d dc b", d=128))

    # q[h, b] = sum_d W_wT[d, h] * decT[d, b]  (fp32)
    qp = ptp.tile([128, BL], F32, tag="tp")
    for dc in range(ND):
        nc.tensor.matmul(
            qp, lhsT=wwT[:, dc, :], rhs=decT_sb[:, dc, :],
            start=(dc == 0), stop=(dc == ND - 1),
        )
    vb_sb = consts.tile([128, 1], F32)
    nc.sync.dma_start(out=vb_sb, in_=V_b)
    qvb = consts.tile([128, BL], F32)
    nc.vector.tensor_scalar_add(qvb, qp, vb_sb)

    # w_w as fp16 column [128, 1]
    w_sb = consts.tile([128, 1], F16)
    nc.sync.dma_start(out=w_sb, in_=w_in)

    aps["io"] = (enc, encT, maskT_in, out_ctx, out_aw)
    aps["w"] = (ident, vwT, qvb, w_sb, ones, ones_row, negfour)

    # ---- software-pipelined batches ----
    # per iteration: softmax(b-1) first (so exp is not queued behind the
    # next batch's tanh ops on ACT), then frontend(b), then the PE half of
    # backend(b-1), then the last erg groups of b.
    w_sbf = aps["w"][3]
    prev = None
    sm = None
    for b in range(BL):
        if prev is not None:
            sm = _softmax(nc, pools, aps, b - 1, *prev)
        cur = _frontend(nc, pools, aps, b)
        if prev is not None:
            _pe_backend(nc, pools, aps, b - 1, prev[0], *sm)
        erg_ps_b, tanh_ts_b = cur[1], cur[3]
        _erg_group(nc, erg_ps_b, tanh_ts_b[NG - 2], w_sbf, NG - 2)
        _erg_group(nc, erg_ps_b, tanh_ts_b[NG - 1], w_sbf, NG - 1)
        prev = cur
    sm = _softmax(nc, pools, aps, BL - 1, *prev)
    _pe_backend(nc, pools, aps, BL - 1, prev[0], *sm)


_CACHE = {}


def _get_nc():
    if "nc" in _CACHE:
        return _CACHE["nc"]
    nc = bacc.Bacc("TRN2", target_bir_lowering=False, debug=False)
    enc = nc.dram_tensor("enc", [BL, T, E], F16, kind="ExternalInput").ap()
    encT = nc.dram_tensor("encT", [BL, E, T], F16, kind="ExternalInput").ap()
    maskT = nc.dram_tensor("maskT", [BL, 128, NC_T], F32, kind="ExternalInput").ap()
    decT = nc.dram_tensor("decT", [D, BL], F32, kind="ExternalInput").ap()
    V_wT = nc.dram_tensor("V_wT", [E, H], F16, kind="ExternalInput").ap()
    V_b = nc.dram_tensor("V_b", [H, 1], F32, kind="ExternalInput").ap()
    W_wT = nc.dram_tensor("W_wT", [D, H], F32, kind="ExternalInput").ap()
    w_col = nc.dram_tensor("w_col", [H, 1], F16, kind="ExternalInput").ap()
    out_ctx = nc.dram_tensor("out_ctx", [BL, E], F32, kind="ExternalOutput").ap()
    out_aw = nc.dram_tensor("out_aw", [BL, T], F32, kind="ExternalOutput").ap()
    aps = {"dram": (enc, encT, maskT, decT, V_b, W_wT, w_col, out_ctx, out_aw)}
    aps["vwt_dram"] = V_wT
    with tile.TileContext(nc) as tc:
        with ExitStack() as ctx:
            build_kernel(ctx, tc, aps)
    nc.compile()
    _CACHE["nc"] = nc
    return nc


def make_in_maps(encoder_outs, decoder_state, mask, V_w, V_b, W_w, w_w):
    enc = np.asarray(encoder_outs, dtype=np.float32)
    enc16 = enc.astype(np.float16)
    # encT columns permuted so free position i = 128*c + p holds t = 16*p + c
    encTn = enc.swapaxes(1, 2).astype(np.float16)          # [B, E, T]
    encT16 = np.ascontiguousarray(
        encTn.reshape(B, E, 128, NC_T).transpose(0, 1, 3, 2).reshape(B, E, T)
    )
    m = np.asarray(mask).astype(np.float32) * NEG - 4.0    # [B, T]
    maskT = np.ascontiguousarray(m.reshape(B, 128, NC_T))  # [b, p, c], t=16p+c
    decT = np.ascontiguousarray(np.asarray(decoder_state, np.float32).T)  # [D, B]
    V_wT = np.ascontiguousarray(np.asarray(V_w, np.float32).T).astype(np.float16)
    V_b_c = np.asarray(V_b, np.float32).reshape(H, 1)
    W_wT = np.ascontiguousarray(np.asarray(W_w, np.float32).T)  # [D, H]
    w_col = np.asarray(w_w, np.float32).reshape(1, H).T.astype(np.float16)
    w_col = np.ascontiguousarray(w_col)
    in_maps = []
    for c in range(NCORES):
        sl = slice(c * BL, (c + 1) * BL)
        in_maps.append(
            {
                "enc": enc16[sl],
                "encT": encT16[sl],
                "maskT": maskT[sl],
                "decT": decT[:, sl],
                "V_wT": V_wT,
                "V_b": V_b_c,
                "W_wT": W_wT,
                "w_col": w_col,
            }
        )
    return in_maps


def run(in_maps, trace=False, **kw):
    nc = _get_nc()
    return run_bass_kernel_spmd(nc, in_maps, list(range(NCORES)), trace=trace, **kw)


def kernel(encoder_outs, decoder_state, mask, V_w, V_b, W_w, w_w, w_b=None):
    in_maps = make_in_maps(encoder_outs, decoder_state, mask, V_w, V_b, W_w, w_w)
    res = run(in_maps)
    ctx_u = np.concatenate([r["out_ctx"] for r in res.results], axis=0)
    aw_u = np.concatenate([r["out_aw"] for r in res.results], axis=0)
    # device rows are [c, p] with t = 16p + c; undo the permutation
    aw_u = aw_u.reshape(B, NC_T, 128).swapaxes(1, 2).reshape(B, T)
    s = aw_u.sum(axis=1, keepdims=True)
    return (ctx_u / s).astype(np.float32), (aw_u / s).astype(np.float32)


# revision 20
# speedup vs baseline: 1.1020x; 1.1020x over previous
"""Bahdanau attention kernel for Trainium2 (8 NeuronCores, batch-parallel).

Computes, for B=64, T=2048, E=512, H=128, D=1024:
    q   = dec @ W_w.T                      [B, H]
    k   = enc @ V_w.T + V_b                [B, T, H]
    erg = tanh(q + k) @ w_w[0] (+ w_b)     [B, T]
    erg = where(mask, -inf, erg)
    aw  = softmax(erg, axis=1)             [B, T]
    ctx = einsum('bte,bt->be', enc, aw)    [B, E]
returns (ctx, aw).

Sharding: data-parallel over batch, 8 batches per core; weights replicated.
w_b is dropped (softmax is shift-invariant). Softmax skips max-subtraction:
|erg| <= sum|w_w| ~ 11.4, so exp never overflows; masked entries get -1e30
which underflows exp to exactly 0.

The host ships encoder_outs twice in fp16 — natural [t, e] (context rhs) and
pre-transposed [e, t] (projection rhs) — same HBM bytes as one fp32 copy.
This removes all on-chip 128x128 PE transposes of enc (the dominant cost of
the v1 kernel: each matmul carries a ~125ns LDWEIGHTS + drain overhead, and
transposes were 512 of ~1100 matmuls). Small weights (V_wT, W_wT, decT, w)
and the additive mask (pre-scaled to -1e30, transposed to [t%128, c]) are
also pre-arranged on the host.

Per-core dataflow (batch b local):
  - k[h=128p, 512t] = sum_ec V_wT_f16[:,ec].T @ encT_f16[:,ec,g] (fp16, N=512)
  - tanh fused with per-partition bias q[h]+V_b[h] on ACT, PSUM source.
  - erg.T[t=128p, 1] per 128-t block via matmul(lhsT=tanh_f16_block, rhs=w_f16)
  - masked softmax on [128, 16] (t%128 x t//128) tiles; partition sums via
    ones-matmuls on PE (cheap, N=1).
  - ctx[1, 512] accumulated over 16 chunks: matmul(lhsT=alpha_f16_col,
    rhs=enc_f16_chunk).
  - aw output via PE transpose of alpha (fp32r) back to [c, t%128] rows.
Batches are software-pipelined: frontend(b) then backend(b-1).
"""

import numpy as np
from contextlib import ExitStack

import concourse.bacc as bacc
import concourse.bass as bass
import concourse.tile as tile
from concourse import mybir
from concourse.bass_utils import run_bass_kernel_spmd
from concourse.masks import make_identity

F32 = mybir.dt.float32
F32R = mybir.dt.float32r
F16 = mybir.dt.float16

B, T, E, H, D = 64, 2048, 512, 128, 1024
NCORES = 8
BL = B // NCORES          # 8 local batches per core
NC_T = T // 128           # 16 chunks of 128 along t
NG = T // 512             # 4 groups of 512 along t
NE = E // 128             # 4 e-chunks
ND = D // 128             # 8 d-chunks
NEG = -1.0e30


def _frontend(nc, pools, aps, b):
    """Load enc[b]/encT[b]/maskT[b]; project, tanh, erg -> erg_psum[b]."""
    (consts, encp, tanhp, smalls, ptp, pkp, pergp, _pctxp) = pools
    enc, encT, maskT_in, _oc, _oa = aps["io"]
    ident, vwT, qvb, w_sb, ones, ones_row, negfour = aps["w"]

    # t index mapping everywhere on-chip: t = 16*p + c (p = partition,
    # c = chunk). enc rows for one partition are then 16 KB contiguous in
    # DRAM (full-rate DMA descriptors); encT is shipped with its t columns
    # pre-permuted to match, so per-partition runs are 4 KB.
    enc_t = encp.tile([128, NC_T, E], F16, tag="enc", name=f"enc_t{b}")
    encT_t = encp.tile([128, NE, T], F16, tag="encT", name=f"encT_t{b}")
    encT_src = encT[b].rearrange("(ec p) t -> p ec t", p=128)
    for ec in range(NE):
        nc.sync.dma_start(out=encT_t[:, ec, :], in_=encT_src[:, ec, :])
    nc.sync.dma_start(out=enc_t, in_=enc[b].rearrange("(p c) e -> p c e", c=NC_T))

    maskT = smalls.tile([128, NC_T], F32, tag="maskT", name=f"maskT{b}")
    nc.gpsimd.dma_start(out=maskT, in_=maskT_in[b])

    erg_ps = pergp.tile([128, NC_T], F32, tag="ergp", name=f"erg_ps{b}")
    tanh_ts = []
    for g in range(NG):
        # k[h, 512t] accumulated over e-chunks (fp16, N=512)
        kp = pkp.tile([128, 512], F32, tag="kp", name=f"kp{b}_{g}")
        for ec in range(NE):
            nc.tensor.matmul(
                kp,
                lhsT=vwT[:, ec, :],
                rhs=encT_t[:, ec, 512 * g : 512 * (g + 1)],
                start=(ec == 0),
                stop=(ec == NE - 1),
            )
        # tanh(k + q[h] + V_b[h]) on ACT straight from PSUM, fp16 out
        tanh_t = tanhp.tile([128, 512], F16, tag="tanh", name=f"tanh{b}_{g}")
        nc.scalar.activation(
            tanh_t, kp, mybir.ActivationFunctionType.Tanh, bias=qvb[:, b : b + 1]
        )
        tanh_ts.append(tanh_t)

        # erg lags two groups behind so tanh (ACT) is never on the PE
        # critical path; the last two groups are emitted by the caller
        # after backend(b-1)'s matmuls.
        if g >= 2:
            _erg_group(nc, erg_ps, tanh_ts[g - 2], w_sb, g - 2)

    return enc_t, erg_ps, maskT, tanh_ts


def _erg_group(nc, erg_ps, tanh_t, w_sb, g):
    for tb in range(4):
        col = 4 * g + tb
        nc.tensor.matmul(
            erg_ps[:, col : col + 1],
            lhsT=tanh_t[:, 128 * tb : 128 * (tb + 1)],
            rhs=w_sb,
            start=True,
            stop=True,
        )


def _softmax(nc, pools, aps, b, enc_t, erg_ps, maskT, tanh_ts):
    """Unnormalized masked softmax of batch b (DVE/ACT only): the -4 shift
    (folded into the host-side mask) keeps fp16 in range and cancels in the
    host-side division by the row sum."""
    (consts, encp, tanhp, smalls, ptp, pkp, pergp, pctxp) = pools
    ergm = smalls.tile([128, NC_T], F32, tag="ergm", name=f"ergm{b}")
    nc.vector.tensor_add(ergm, erg_ps, maskT)
    e_sb = smalls.tile([128, NC_T], F32, tag="e_sb", name=f"e_sb{b}")
    nc.scalar.activation(e_sb, ergm, mybir.ActivationFunctionType.Exp)
    alpha = smalls.tile([128, NC_T], F32R, tag="alpha", name=f"alpha{b}")
    nc.vector.tensor_scalar_mul(alpha, e_sb, 1.0)
    alpha16 = smalls.tile([128, NC_T], F16, tag="alpha16", name=f"alpha16_{b}")
    nc.vector.tensor_scalar_mul(alpha16, e_sb, 1.0)
    return alpha, alpha16


def _pe_backend(nc, pools, aps, b, enc_t, alpha, alpha16):
    """context(b) and outputs(b)."""
    (consts, encp, tanhp, smalls, ptp, pkp, pergp, pctxp) = pools
    _enc, _encT, _m, out_ctx, out_aw = aps["io"]
    ident, vwT, qvb, w_sb, ones, ones_row, negfour = aps["w"]

    # context: ctx[1, 512] += alpha_chunk.T @ enc_chunk over 16 chunks (fp16)
    ctx_ps = pctxp.tile([1, E], F32, tag="ctxp", name=f"ctx_ps{b}")
    for c in range(NC_T):
        nc.tensor.matmul(
            ctx_ps,
            lhsT=alpha16[:, c : c + 1],
            rhs=enc_t[:, c, :],
            start=(c == 0),
            stop=(c == NC_T - 1),
        )
    ctx_sb = smalls.tile([1, E], F32, tag="ctx_sb", name=f"ctx_sb{b}")
    nc.vector.tensor_copy(ctx_sb, ctx_ps)
    nc.scalar.dma_start(out=out_ctx[b : b + 1, :], in_=ctx_sb)

    # attention weights out: transpose alpha back to [c, t%128] rows (fp32r)
    et_ps = ptp.tile([16, 128], F32R, tag="tp", name=f"et_ps{b}")
    nc.tensor.transpose(et_ps, alpha, ident)
    aw_sb = smalls.tile([16, 128], F32, tag="aw_sb", name=f"aw_sb{b}")
    nc.vector.tensor_copy(aw_sb, et_ps.bitcast(F32))
    nc.scalar.dma_start(out=out_aw[b].rearrange("(c p) -> c p", p=128), in_=aw_sb)


def build_kernel(ctx, tc, aps):
    nc = tc.nc
    (enc, encT, maskT_in, decT, V_b, W_wT, w_in, out_ctx, out_aw) = aps["dram"]

    consts = ctx.enter_context(tc.tile_pool(name="consts", bufs=1))
    encp = ctx.enter_context(tc.tile_pool(name="encp", bufs=4))
    tanhp = ctx.enter_context(tc.tile_pool(name="tanhp", bufs=3))
    smalls = ctx.enter_context(tc.tile_pool(name="smalls", bufs=2))
    ptp = ctx.enter_context(tc.tile_pool(name="ptp", bufs=2, space="PSUM"))
    pkp = ctx.enter_context(tc.tile_pool(name="pkp", bufs=3, space="PSUM"))
    pergp = ctx.enter_context(tc.tile_pool(name="pergp", bufs=2, space="PSUM"))
    pctxp = ctx.enter_context(tc.tile_pool(name="pctxp", bufs=1, space="PSUM"))
    pools = (consts, encp, tanhp, smalls, ptp, pkp, pergp, pctxp)

    # ---- constants / setup ----
    ident32 = consts.tile([128, 128], F32)
    make_identity(nc, ident32)
    # gpsimd can't write f32r; round the identity through a DVE copy instead
    ident = consts.tile([128, 128], F32R)
    nc.vector.tensor_copy(ident, ident32)

    ones = consts.tile([128, 1], F32)
    nc.vector.memset(ones, 1.0)
    ones_row = consts.tile([1, 128], F32)
    nc.vector.memset(ones_row, 1.0)
    negfour = consts.tile([128, 1], F32)
    nc.vector.memset(negfour, -4.0)

    # host-pretransposed weights: V_wT [e, ec, h] fp16
    vwT = consts.tile([128, NE, 128], F16)
    nc.sync.dma_start(
        out=vwT, in_=aps["vwt_dram"].rearrange("(ec e) h -> e ec h", e=128)
    )

    # W_wT [d, dc, h] f32 and decT [d, dc, b] f32 for the q projection
    wwT = consts.tile([128, ND, 128], F32)
    nc.sync.dma_start(out=wwT, in_=W_wT.rearrange("(dc d) h -> d dc h", d=128))
    decT_sb = consts.tile([128, ND, BL], F32)
    nc.sync.dma_start(out=decT_sb, in_=decT.rearrange("(dc d) b -> d dc b", d=128))

    # q[h, b] = sum_d W_wT[d, h] * decT[d, b]  (fp32)
    qp = ptp.tile([128, BL], F32, tag="tp")
    for dc in range(ND):
        nc.tensor.matmul(
            qp, lhsT=wwT[:, dc, :], rhs=decT_sb[:, dc, :],
            start=(dc == 0), stop=(dc == ND - 1),
        )
    vb_sb = consts.tile([128, 1], F32)
    nc.sync.dma_start(out=vb_sb, in_=V_b)
    qvb = consts.tile([128, BL], F32)
    nc.vector.tensor_scalar_add(qvb, qp, vb_sb)

    # w_w as fp16 column [128, 1]
    w_sb = consts.tile([128, 1], F16)
    nc.sync.dma_start(out=w_sb, in_=w_in)

    aps["io"] = (enc, encT, maskT_in, out_ctx, out_aw)
    aps["w"] = (ident, vwT, qvb, w_sb, ones, ones_row, negfour)

    # ---- software-pipelined batches ----
    # per iteration: softmax(b-1) first (so exp is not queued behind the
    # next batch's tanh ops on ACT), then frontend(b), then the PE half of
    # backend(b-1), then the last erg groups of b.
    w_sbf = aps["w"][3]
    prev = None
    sm = None
    for b in range(BL):
        if prev is not None:
            sm = _softmax(nc, pools, aps, b - 1, *prev)
        cur = _frontend(nc, pools, aps, b)
        if prev is not None:
            _pe_backend(nc, pools, aps, b - 1, prev[0], *sm)
        erg_ps_b, tanh_ts_b = cur[1], cur[3]
        _erg_group(nc, erg_ps_b, tanh_ts_b[NG - 2], w_sbf, NG - 2)
        _erg_group(nc, erg_ps_b, tanh_ts_b[NG - 1], w_sbf, NG - 1)
        prev = cur
    sm = _softmax(nc, pools, aps, BL - 1, *prev)
    _pe_backend(nc, pools, aps, BL - 1, prev[0], *sm)


_CACHE = {}


def _get_nc():
    if "nc" in _CACHE:
        return _CACHE["nc"]
    nc = bacc.Bacc("TRN2", target_bir_lowering=False, debug=False)
    enc = nc.dram_tensor("enc", [BL, T, E], F16, kind="ExternalInput").ap()
    encT = nc.dram_tensor("encT", [BL, E, T], F16, kind="ExternalInput").ap()
    maskT = nc.dram_tensor("maskT", [BL, 128, NC_T], F32, kind="ExternalInput").ap()
    decT = nc.dram_tensor("decT", [D, BL], F32, kind="ExternalInput").ap()
    V_wT = nc.dram_tensor("V_wT", [E, H], F16, kind="ExternalInput").ap()
    V_b = nc.dram_tensor("V_b", [H, 1], F32, kind="ExternalInput").ap()
    W_wT = nc.dram_tensor("W_wT", [D, H], F32, kind="ExternalInput").ap()
    w_col = nc.dram_tensor("w_col", [H, 1], F16, kind="ExternalInput").ap()
    out_ctx = nc.dram_tensor("out_ctx", [BL, E], F32, kind="ExternalOutput").ap()
    out_aw = nc.dram_tensor("out_aw", [BL, T], F32, kind="ExternalOutput").ap()
    aps = {"dram": (enc, encT, maskT, decT, V_b, W_wT, w_col, out_ctx, out_aw)}
    aps["vwt_dram"] = V_wT
    with tile.TileContext(nc) as tc:
        with ExitStack() as ctx:
            build_kernel(ctx, tc, aps)
    nc.compile()
    _CACHE["nc"] = nc
    return nc


def make_in_maps(encoder_outs, decoder_state, mask, V_w, V_b, W_w, w_w):
    enc = np.asarray(encoder_outs, dtype=np.float32)
    enc16 = enc.astype(np.float16)
    # encT columns permuted so free position i = 128*c + p holds t = 16*p + c
    encTn = enc.swapaxes(1, 2).astype(np.float16)          # [B, E, T]
    encT16 = np.ascontiguousarray(
        encTn.reshape(B, E, 128, NC_T).transpose(0, 1, 3, 2).reshape(B, E, T)
    )
    m = np.asarray(mask).astype(np.float32) * NEG - 4.0    # [B, T]
    maskT = np.ascontiguousarray(m.reshape(B, 128, NC_T))  # [b, p, c], t=16p+c
    decT = np.ascontiguousarray(np.asarray(decoder_state, np.float32).T)  # [D, B]
    V_wT = np.ascontiguousarray(np.asarray(V_w, np.float32).T).astype(np.float16)
    V_b_c = np.asarray(V_b, np.float32).reshape(H, 1)
    W_wT = np.ascontiguousarray(np.asarray(W_w, np.float32).T)  # [D, H]
    w_col = np.asarray(w_w, np.float32).reshape(1, H).T.astype(np.float16)
    w_col = np.ascontiguousarray(w_col)
    in_maps = []
    for c in range(NCORES):
        sl = slice(c * BL, (c + 1) * BL)
        in_maps.append(
            {
                "enc": enc16[sl],
                "encT": encT16[sl],
                "maskT": maskT[sl],
                "decT": decT[:, sl],
                "V_wT": V_wT,
                "V_b": V_b_c,
                "W_wT": W_wT,
                "w_col": w_col,
            }
        )
    return in_maps


def run(in_maps, trace=False, **kw):
    nc = _get_nc()
    return run_bass_kernel_spmd(nc, in_maps, list(range(NCORES)), trace=trace, **kw)


def kernel(encoder_outs, decoder_state, mask, V_w, V_b, W_w, w_w, w_b=None):
    in_maps = make_in_maps(encoder_outs, decoder_state, mask, V_w, V_b, W_w, w_w)
    res = run(in_maps)
    ctx_u = np.concatenate([r["out_ctx"] for r in res.results], axis=0)
    aw_u = np.concatenate([r["out_aw"] for r in res.results], axis=0)
    # device rows are [c, p] with t = 16p + c; undo the permutation
    aw_u = aw_u.reshape(B, NC_T, 128).swapaxes(1, 2).reshape(B, T)
    s = aw_u.sum(axis=1, keepdims=True)
    return (ctx_u / s).astype(np.float32), (aw_u / s).astype(np.float32)


# revision 21
# speedup vs baseline: 1.1304x; 1.0258x over previous
"""Bahdanau attention kernel for Trainium2 (8 NeuronCores, batch-parallel).

Computes, for B=64, T=2048, E=512, H=128, D=1024:
    q   = dec @ W_w.T                      [B, H]
    k   = enc @ V_w.T + V_b                [B, T, H]
    erg = tanh(q + k) @ w_w[0] (+ w_b)     [B, T]
    erg = where(mask, -inf, erg)
    aw  = softmax(erg, axis=1)             [B, T]
    ctx = einsum('bte,bt->be', enc, aw)    [B, E]
returns (ctx, aw).

Sharding: data-parallel over batch, 8 batches per core; weights replicated.
w_b is dropped (softmax is shift-invariant). Softmax skips max-subtraction:
|erg| <= sum|w_w| ~ 11.4, so exp never overflows; masked entries get -1e30
which underflows exp to exactly 0.

The host ships encoder_outs twice in fp16 — natural [t, e] (context rhs) and
pre-transposed [e, t] (projection rhs) — same HBM bytes as one fp32 copy.
This removes all on-chip 128x128 PE transposes of enc (the dominant cost of
the v1 kernel: each matmul carries a ~125ns LDWEIGHTS + drain overhead, and
transposes were 512 of ~1100 matmuls). Small weights (V_wT, W_wT, decT, w)
and the additive mask (pre-scaled to -1e30, transposed to [t%128, c]) are
also pre-arranged on the host.

Per-core dataflow (batch b local):
  - k[h=128p, 512t] = sum_ec V_wT_f16[:,ec].T @ encT_f16[:,ec,g] (fp16, N=512)
  - tanh fused with per-partition bias q[h]+V_b[h] on ACT, PSUM source.
  - erg.T[t=128p, 1] per 128-t block via matmul(lhsT=tanh_f16_block, rhs=w_f16)
  - masked softmax on [128, 16] (t%128 x t//128) tiles; partition sums via
    ones-matmuls on PE (cheap, N=1).
  - ctx[1, 512] accumulated over 16 chunks: matmul(lhsT=alpha_f16_col,
    rhs=enc_f16_chunk).
  - aw output via PE transpose of alpha (fp32r) back to [c, t%128] rows.
Batches are software-pipelined: frontend(b) then backend(b-1).
"""

import numpy as np
from contextlib import ExitStack

import concourse.bacc as bacc
import concourse.bass as bass
import concourse.tile as tile
from concourse import mybir
from concourse.bass_utils import run_bass_kernel_spmd
from concourse.masks import make_identity

F32 = mybir.dt.float32
F32R = mybir.dt.float32r
F16 = mybir.dt.float16

B, T, E, H, D = 64, 2048, 512, 128, 1024
NCORES = 8
BL = B // NCORES          # 8 local batches per core
NC_T = T // 128           # 16 chunks of 128 along t
NG = T // 512             # 4 groups of 512 along t
NE = E // 128             # 4 e-chunks
ND = D // 128             # 8 d-chunks
NEG = -1.0e30


def _frontend(nc, pools, aps, b):
    """Load enc[b]/encT[b]/maskT[b]; project, tanh, erg -> erg_psum[b]."""
    (consts, encp, tanhp, smalls, ptp, pkp, pergp, _pctxp) = pools
    enc, encT, maskT_in, _oc, _oa = aps["io"]
    ident, vwT, qvb, w_sb, ones, ones_row, negfour = aps["w"]

    # t index mapping everywhere on-chip: t = 16*p + c (p = partition,
    # c = chunk). enc rows for one partition are then 16 KB contiguous in
    # DRAM (full-rate DMA descriptors); encT is shipped with its t columns
    # pre-permuted to match, so per-partition runs are 4 KB.
    enc_t = encp.tile([128, NC_T, E], F16, tag="enc", name=f"enc_t{b}")
    encT_t = encp.tile([128, NE, T], F16, tag="encT", name=f"encT_t{b}")
    encT_src = encT[b].rearrange("(ec p) t -> p ec t", p=128)
    if b == 0:
        # halves: the first k-projection only needs the first 1024 t columns,
        # so it can start after 2 MB instead of 4 MB of encT
        for half in range(2):
            for ec in range(NE):
                nc.sync.dma_start(
                    out=encT_t[:, ec, 1024 * half : 1024 * (half + 1)],
                    in_=encT_src[:, ec, 1024 * half : 1024 * (half + 1)],
                )
    else:
        for ec in range(NE):
            nc.sync.dma_start(out=encT_t[:, ec, :], in_=encT_src[:, ec, :])
    nc.sync.dma_start(out=enc_t, in_=enc[b].rearrange("(p c) e -> p c e", c=NC_T))

    maskT = smalls.tile([128, NC_T], F32, tag="maskT", name=f"maskT{b}")
    nc.gpsimd.dma_start(out=maskT, in_=maskT_in[b])

    erg_ps = pergp.tile([128, NC_T], F32, tag="ergp", name=f"erg_ps{b}")
    tanh_ts = []
    for g in range(NG):
        # k[h, 512t] accumulated over e-chunks (fp16, N=512)
        kp = pkp.tile([128, 512], F32, tag="kp", name=f"kp{b}_{g}")
        for ec in range(NE):
            nc.tensor.matmul(
                kp,
                lhsT=vwT[:, ec, :],
                rhs=encT_t[:, ec, 512 * g : 512 * (g + 1)],
                start=(ec == 0),
                stop=(ec == NE - 1),
            )
        # tanh(k + q[h] + V_b[h]) on ACT straight from PSUM, fp16 out
        tanh_t = tanhp.tile([128, 512], F16, tag="tanh", name=f"tanh{b}_{g}")
        nc.scalar.activation(
            tanh_t, kp, mybir.ActivationFunctionType.Tanh, bias=qvb[:, b : b + 1]
        )
        tanh_ts.append(tanh_t)

        # erg lags two groups behind so tanh (ACT) is never on the PE
        # critical path; the last two groups are emitted by the caller
        # after backend(b-1)'s matmuls.
        if g >= 2:
            _erg_group(nc, erg_ps, tanh_ts[g - 2], w_sb, g - 2)

    return enc_t, erg_ps, maskT, tanh_ts


def _erg_group(nc, erg_ps, tanh_t, w_sb, g):
    for tb in range(4):
        col = 4 * g + tb
        nc.tensor.matmul(
            erg_ps[:, col : col + 1],
            lhsT=tanh_t[:, 128 * tb : 128 * (tb + 1)],
            rhs=w_sb,
            start=True,
            stop=True,
        )


def _softmax(nc, pools, aps, b, enc_t, erg_ps, maskT, tanh_ts):
    """Unnormalized masked softmax of batch b (DVE/ACT only): the -4 shift
    (folded into the host-side mask) keeps fp16 in range and cancels in the
    host-side division by the row sum."""
    (consts, encp, tanhp, smalls, ptp, pkp, pergp, pctxp) = pools
    ergm = smalls.tile([128, NC_T], F32, tag="ergm", name=f"ergm{b}")
    nc.vector.tensor_add(ergm, erg_ps, maskT)
    e_sb = smalls.tile([128, NC_T], F32, tag="e_sb", name=f"e_sb{b}")
    nc.scalar.activation(e_sb, ergm, mybir.ActivationFunctionType.Exp)
    alpha = smalls.tile([128, NC_T], F32R, tag="alpha", name=f"alpha{b}")
    nc.vector.tensor_scalar_mul(alpha, e_sb, 1.0)
    alpha16 = smalls.tile([128, NC_T], F16, tag="alpha16", name=f"alpha16_{b}")
    nc.vector.tensor_scalar_mul(alpha16, e_sb, 1.0)
    return alpha, alpha16


def _pe_backend(nc, pools, aps, b, enc_t, alpha, alpha16):
    """context(b) and outputs(b)."""
    (consts, encp, tanhp, smalls, ptp, pkp, pergp, pctxp) = pools
    _enc, _encT, _m, out_ctx, out_aw = aps["io"]
    ident, vwT, qvb, w_sb, ones, ones_row, negfour = aps["w"]

    # context: ctx[1, 512] += alpha_chunk.T @ enc_chunk over 16 chunks (fp16)
    ctx_ps = pctxp.tile([1, E], F32, tag="ctxp", name=f"ctx_ps{b}")
    for c in range(NC_T):
        nc.tensor.matmul(
            ctx_ps,
            lhsT=alpha16[:, c : c + 1],
            rhs=enc_t[:, c, :],
            start=(c == 0),
            stop=(c == NC_T - 1),
        )
    ctx_sb = smalls.tile([1, E], F32, tag="ctx_sb", name=f"ctx_sb{b}")
    nc.vector.tensor_copy(ctx_sb, ctx_ps)
    nc.scalar.dma_start(out=out_ctx[b : b + 1, :], in_=ctx_sb)

    # attention weights out: transpose alpha back to [c, t%128] rows (fp32r)
    et_ps = ptp.tile([16, 128], F32R, tag="tp", name=f"et_ps{b}")
    nc.tensor.transpose(et_ps, alpha, ident)
    aw_sb = smalls.tile([16, 128], F32, tag="aw_sb", name=f"aw_sb{b}")
    nc.vector.tensor_copy(aw_sb, et_ps.bitcast(F32))
    nc.scalar.dma_start(out=out_aw[b].rearrange("(c p) -> c p", p=128), in_=aw_sb)


def build_kernel(ctx, tc, aps):
    nc = tc.nc
    (enc, encT, maskT_in, decT, V_b, W_wT, w_in, out_ctx, out_aw) = aps["dram"]

    consts = ctx.enter_context(tc.tile_pool(name="consts", bufs=1))
    encp = ctx.enter_context(tc.tile_pool(name="encp", bufs=4))
    tanhp = ctx.enter_context(tc.tile_pool(name="tanhp", bufs=3))
    smalls = ctx.enter_context(tc.tile_pool(name="smalls", bufs=2))
    ptp = ctx.enter_context(tc.tile_pool(name="ptp", bufs=2, space="PSUM"))
    pkp = ctx.enter_context(tc.tile_pool(name="pkp", bufs=3, space="PSUM"))
    pergp = ctx.enter_context(tc.tile_pool(name="pergp", bufs=2, space="PSUM"))
    pctxp = ctx.enter_context(tc.tile_pool(name="pctxp", bufs=1, space="PSUM"))
    pools = (consts, encp, tanhp, smalls, ptp, pkp, pergp, pctxp)

    # ---- constants / setup ----
    ident32 = consts.tile([128, 128], F32)
    make_identity(nc, ident32)
    # gpsimd can't write f32r; round the identity through a DVE copy instead
    ident = consts.tile([128, 128], F32R)
    nc.vector.tensor_copy(ident, ident32)

    ones = consts.tile([128, 1], F32)
    nc.vector.memset(ones, 1.0)
    ones_row = consts.tile([1, 128], F32)
    nc.vector.memset(ones_row, 1.0)
    negfour = consts.tile([128, 1], F32)
    nc.vector.memset(negfour, -4.0)

    # host-pretransposed weights: V_wT [e, ec, h] fp16
    vwT = consts.tile([128, NE, 128], F16)
    nc.scalar.dma_start(
        out=vwT, in_=aps["vwt_dram"].rearrange("(ec e) h -> e ec h", e=128)
    )

    # W_wT [d, dc, h] f32 and decT [d, dc, b] f32 for the q projection
    wwT = consts.tile([128, ND, 128], F32)
    nc.scalar.dma_start(out=wwT, in_=W_wT.rearrange("(dc d) h -> d dc h", d=128))
    decT_sb = consts.tile([128, ND, BL], F32)
    nc.scalar.dma_start(out=decT_sb, in_=decT.rearrange("(dc d) b -> d dc b", d=128))

    # q[h, b] = sum_d W_wT[d, h] * decT[d, b]  (fp32)
    qp = ptp.tile([128, BL], F32, tag="tp")
    for dc in range(ND):
        nc.tensor.matmul(
            qp, lhsT=wwT[:, dc, :], rhs=decT_sb[:, dc, :],
            start=(dc == 0), stop=(dc == ND - 1),
        )
    vb_sb = consts.tile([128, 1], F32)
    nc.scalar.dma_start(out=vb_sb, in_=V_b)
    qvb = consts.tile([128, BL], F32)
    nc.vector.tensor_scalar_add(qvb, qp, vb_sb)

    # w_w as fp16 column [128, 1]
    w_sb = consts.tile([128, 1], F16)
    nc.scalar.dma_start(out=w_sb, in_=w_in)

    aps["io"] = (enc, encT, maskT_in, out_ctx, out_aw)
    aps["w"] = (ident, vwT, qvb, w_sb, ones, ones_row, negfour)

    # ---- software-pipelined batches ----
    # per iteration: softmax(b-1) first (so exp is not queued behind the
    # next batch's tanh ops on ACT), then frontend(b), then the PE half of
    # backend(b-1), then the last erg groups of b.
    w_sbf = aps["w"][3]
    prev = None
    sm = None
    for b in range(BL):
        if prev is not None:
            sm = _softmax(nc, pools, aps, b - 1, *prev)
        cur = _frontend(nc, pools, aps, b)
        if prev is not None:
            _pe_backend(nc, pools, aps, b - 1, prev[0], *sm)
        erg_ps_b, tanh_ts_b = cur[1], cur[3]
        _erg_group(nc, erg_ps_b, tanh_ts_b[NG - 2], w_sbf, NG - 2)
        _erg_group(nc, erg_ps_b, tanh_ts_b[NG - 1], w_sbf, NG - 1)
        prev = cur
    sm = _softmax(nc, pools, aps, BL - 1, *prev)
    _pe_backend(nc, pools, aps, BL - 1, prev[0], *sm)


_CACHE = {}


def _get_nc():
    if "nc" in _CACHE:
        return _CACHE["nc"]
    nc = bacc.Bacc("TRN2", target_bir_lowering=False, debug=False)
    enc = nc.dram_tensor("enc", [BL, T, E], F16, kind="ExternalInput").ap()
    encT = nc.dram_tensor("encT", [BL, E, T], F16, kind="ExternalInput").ap()
    maskT = nc.dram_tensor("maskT", [BL, 128, NC_T], F32, kind="ExternalInput").ap()
    decT = nc.dram_tensor("decT", [D, BL], F32, kind="ExternalInput").ap()
    V_wT = nc.dram_tensor("V_wT", [E, H], F16, kind="ExternalInput").ap()
    V_b = nc.dram_tensor("V_b", [H, 1], F32, kind="ExternalInput").ap()
    W_wT = nc.dram_tensor("W_wT", [D, H], F32, kind="ExternalInput").ap()
    w_col = nc.dram_tensor("w_col", [H, 1], F16, kind="ExternalInput").ap()
    out_ctx = nc.dram_tensor("out_ctx", [BL, E], F32, kind="ExternalOutput").ap()
    out_aw = nc.dram_tensor("out_aw", [BL, T], F32, kind="ExternalOutput").ap()
    aps = {"dram": (enc, encT, maskT, decT, V_b, W_wT, w_col, out_ctx, out_aw)}
    aps["vwt_dram"] = V_wT
    with tile.TileContext(nc) as tc:
        with ExitStack() as ctx:
            build_kernel(ctx, tc, aps)
    nc.compile()
    _CACHE["nc"] = nc
    return nc


def make_in_maps(encoder_outs, decoder_state, mask, V_w, V_b, W_w, w_w):
    enc = np.asarray(encoder_outs, dtype=np.float32)
    enc16 = enc.astype(np.float16)
    # encT columns permuted so free position i = 128*c + p holds t = 16*p + c
    encTn = enc.swapaxes(1, 2).astype(np.float16)          # [B, E, T]
    encT16 = np.ascontiguousarray(
        encTn.reshape(B, E, 128, NC_T).transpose(0, 1, 3, 2).reshape(B, E, T)
    )
    m = np.asarray(mask).astype(np.float32) * NEG - 4.0    # [B, T]
    maskT = np.ascontiguousarray(m.reshape(B, 128, NC_T))  # [b, p, c], t=16p+c
    decT = np.ascontiguousarray(np.asarray(decoder_state, np.float32).T)  # [D, B]
    V_wT = np.ascontiguousarray(np.asarray(V_w, np.float32).T).astype(np.float16)
    V_b_c = np.asarray(V_b, np.float32).reshape(H, 1)
    W_wT = np.ascontiguousarray(np.asarray(W_w, np.float32).T)  # [D, H]
    w_col = np.asarray(w_w, np.float32).reshape(1, H).T.astype(np.float16)
    w_col = np.ascontiguousarray(w_col)
    in_maps = []
    for c in range(NCORES):
        sl = slice(c * BL, (c + 1) * BL)
        in_maps.append(
            {
                "enc": enc16[sl],
                "encT": encT16[sl],
                "maskT": maskT[sl],
                "decT": decT[:, sl],
                "V_wT": V_wT,
                "V_b": V_b_c,
                "W_wT": W_wT,
                "w_col": w_col,
            }
        )
    return in_maps


def run(in_maps, trace=False, **kw):
    nc = _get_nc()
    return run_bass_kernel_spmd(nc, in_maps, list(range(NCORES)), trace=trace, **kw)


def kernel(encoder_outs, decoder_state, mask, V_w, V_b, W_w, w_w, w_b=None):
    in_maps = make_in_maps(encoder_outs, decoder_state, mask, V_w, V_b, W_w, w_w)
    res = run(in_maps)
    ctx_u = np.concatenate([r["out_ctx"] for r in res.results], axis=0)
    aw_u = np.concatenate([r["out_aw"] for r in res.results], axis=0)
    # device rows are [c, p] with t = 16p + c; undo the permutation
    aw_u = aw_u.reshape(B, NC_T, 128).swapaxes(1, 2).reshape(B, T)
    s = aw_u.sum(axis=1, keepdims=True)
    return (ctx_u / s).astype(np.float32), (aw_u / s).astype(np.float32)
